# revision 44
# baseline (speedup 1.0000x reference)
"""Trainium2 Bass kernel for nn_AdaptiveDecoder (shared MLP + hard-routed type heads).

Strategy:
  * Host: sort nodes by type; split each type's count over 8 cores with minimal
    padding (per-type cap = ceil(count/8) rounded to 4) -> every core sees the
    SAME static layout of type-pure node-column blocks, so the compiled SPMD
    program bakes in the block->head mapping and the device does zero routing.
  * Device: activations stay transposed ([feature, nodes]) so the three matmul
    stages chain without transposes, and the PE runs NOTHING but the GEMMs
    (112 cycles per node column = the bf16 roofline):
      - LayerNorm column sums run on GpSimd (partition_all_reduce over the
        DVE-pre-reduced sum / sum-of-squares tiles).  The Q7 daisy chain
        leaves the result on ALL partitions, so the ACT mean/var chain on
        full [128,nb] tiles yields partition-broadcast -mu and 1/sigma for
        free (engines are partition-parallel: a [128,n] op costs the same as
        [1,n]).
      - The mean correction and 1/sigma scaling fold into two fused DVE ops
        per output chunk: out = ((negmu_b * c2_col) + head_psum) * rsig_b.
        gamma is folded into the head weights host-side; beta/head_b (when
        nonzero) are a per-partition constant added by one more DVE op.
  * All DRAM inputs are pre-tiled on the host into their exact SBUF layouts so
    every load is one dma_start with multi-KB contiguous rows.  This matters
    beyond startup: fine-grained strided input DMAs contend with the PE's rhs
    SBUF streaming port and were measured to stretch matmul pacing 260ns ->
    213ns per 512 columns when fixed.
  * Matmuls run bf16 (full PE rate, half the SBUF traffic of f32r).
  * First/last blocks are split small (256/128 cols) to shrink the block-0
    pipeline-fill bubble and the end-of-kernel LN drain.
"""

import sys

sys.path.insert(0, "/opt/trn_rl_repo")

from contextlib import ExitStack

import numpy as np

N_CORES = 8
LATENT, HIDDEN, OUT, TYPES = 512, 1024, 256, 3
P = 128
NB = 512  # node columns per block (PSUM f32 bank limit)
KL = LATENT // P  # 4 k-tiles, stage 1
KH = HIDDEN // P  # 8 k-tiles, stage 2 / head
MH = HIDDEN // P  # 8 m-chunks of hidden
MO = OUT // P  # 2 m-chunks of head output
LN_EPS = 1e-5
MM_BF16 = True


def _caps_from_counts(counts):
    caps = []
    for tt in range(TYPES):
        cap = -(-int(counts[tt]) // N_CORES)  # ceil
        cap = -(-cap // 4) * 4  # round to 4 cols (keeps DMA rows 8B-aligned)
        caps.append(cap)
    return caps


def _blocks_from_caps(caps):
    """Type-pure blocks tiling [0, R). Remainders split so blocks stay >=256;
    the overall first block is 256 (smaller pipeline-fill bubble) and the
    overall last block is 128 (shorter LN drain after the final matmul)."""
    blocks = []
    off = 0
    for tt in range(TYPES):
        cols = caps[tt]
        j = 0
        while j < cols:
            rem = cols - j
            if rem >= 2 * NB:
                nb = NB
            elif rem > NB:
                nb = -(-((rem + 1) // 2) // 4) * 4
            else:
                nb = rem
            blocks.append((tt, off + j, nb))
            j += nb
        off += cols
    if blocks and blocks[0][2] >= NB:
        t, c0, nb = blocks[0]
        blocks[0:1] = [(t, c0, 256), (t, c0 + 256, nb - 256)]
    if blocks and blocks[-1][2] >= 256:
        t, c0, nb = blocks[-1]
        blocks[-1:] = [(t, c0, nb - 128), (t, c0 + nb - 128, 128)]
    return blocks


def plan(node_types, pad_odd=True):
    """Host-side layout plan shared by all cores.

    Returns (blocks, R, caps, idx_by_type) where idx_by_type[t][c] is the array
    of original row indices of type t assigned to core c.
    """
    node_types = np.asarray(node_types)
    counts = np.bincount(node_types, minlength=TYPES)
    caps = _caps_from_counts(counts)
    idx_by_type = []
    order = np.argsort(node_types, kind="stable")
    starts = np.concatenate([[0], np.cumsum(counts)])
    for tt in range(TYPES):
        idx_t = order[starts[tt] : starts[tt + 1]]
        base, rem = divmod(int(counts[tt]), N_CORES)
        parts, o = [], 0
        for c in range(N_CORES):
            n = base + (1 if c < rem else 0)
            parts.append(idx_t[o : o + n])
            o += n
        idx_by_type.append(parts)
    R = sum(caps)
    blocks = _blocks_from_caps(caps)
    return blocks, R, caps, idx_by_type


def build_program(blocks, R, use_c1=True, mm_bf16=True):
    """blocks: list of (type_idx, col_offset, n_cols); R: node columns per core."""
    import concourse.mybir as mybir
    import concourse.tile as tile
    from concourse import bacc, bass_isa

    dt = mybir.dt
    f32, f32r, bf16 = dt.float32, dt.float32r, dt.bfloat16
    mmdt = bf16 if mm_bf16 else f32r
    AF = mybir.ActivationFunctionType
    ALU = mybir.AluOpType

    nc = bacc.Bacc("TRN2", target_bir_lowering=False, debug=False, num_devices=N_CORES)

    xtd = nc.dram_tensor("xtp", [P, KL * R], mmdt, kind="ExternalInput").ap()
    w1d = nc.dram_tensor("w1p", [P, KL * HIDDEN], mmdt, kind="ExternalInput").ap()
    w2d = nc.dram_tensor("w2p", [P, KH * HIDDEN], mmdt, kind="ExternalInput").ap()
    whpd = nc.dram_tensor("whpp", [P, TYPES * KH * OUT], mmdt, kind="ExternalInput").ap()
    b1d = nc.dram_tensor("b1r", [P, MH], f32, kind="ExternalInput").ap()
    b2d = nc.dram_tensor("b2r", [P, MH], f32, kind="ExternalInput").ap()
    c1d = nc.dram_tensor("c1r", [1, TYPES * OUT], mmdt, kind="ExternalInput").ap()
    c2d = nc.dram_tensor("c2r", [1, TYPES * OUT], mmdt, kind="ExternalInput").ap()
    outd = nc.dram_tensor("out", [OUT, R], f32, kind="ExternalOutput").ap()

    def cv(ap):  # engine-facing view of an mm-dtype tile
        return ap if mm_bf16 else ap.bitcast(f32)

    with tile.TileContext(nc) as tc, ExitStack() as ctx:
        consts = ctx.enter_context(tc.tile_pool(name="consts", bufs=1))
        xt_pool = ctx.enter_context(tc.tile_pool(name="xt", bufs=3))
        h1_pool = ctx.enter_context(tc.tile_pool(name="h1", bufs=2))
        h2_pool = ctx.enter_context(tc.tile_pool(name="h2", bufs=2))
        sq_pool = ctx.enter_context(tc.tile_pool(name="sq", bufs=1))
        hs_pool = ctx.enter_context(tc.tile_pool(name="hs", bufs=2))
        qs_pool = ctx.enter_context(tc.tile_pool(name="qs", bufs=2))
        rv_pool = ctx.enter_context(tc.tile_pool(name="rv", bufs=2))
        ab_pool = ctx.enter_context(tc.tile_pool(name="ab", bufs=2))
        out_pool = ctx.enter_context(tc.tile_pool(name="outp", bufs=2))
        ps_mlp = ctx.enter_context(tc.tile_pool(name="ps_mlp", bufs=3, space="PSUM"))
        ps_head = ctx.enter_context(tc.tile_pool(name="ps_head", bufs=2, space="PSUM"))
        ps_stat = ctx.enter_context(tc.tile_pool(name="ps_stat", bufs=2, space="PSUM"))
        ps_bc = ctx.enter_context(tc.tile_pool(name="ps_bc", bufs=1, space="PSUM"))

        # gpsimd is reserved for the LN partition reductions (plus one-time
        # startup DMAs); recurring xt loads round-robin on the HWDGE queues
        dma_engines = [nc.sync, nc.scalar]
        dma_rr = [0]

        def dma(out, in_):
            eng = dma_engines[dma_rr[0] % len(dma_engines)]
            dma_rr[0] += 1
            eng.dma_start(out=out, in_=in_)

        def load_xt(c0, nb, eng=None):
            xt_t = xt_pool.tile([P, KL * NB], mmdt, tag="xt")
            if eng is None:
                dma(xt_t[:, : KL * nb], xtd[:, KL * c0 : KL * (c0 + nb)])
            else:
                eng.dma_start(out=xt_t[:, : KL * nb], in_=xtd[:, KL * c0 : KL * (c0 + nb)])
            return xt_t

        # --- PE warm-up: the HAM clock-gate needs ~3.4us of sustained PE
        # activity to release full clock; burn the DMA-wait window on dummy
        # matmuls over a memset scratch tile so the first real matmul runs at
        # 2.4 GHz instead of 1.2 ---
        warm_sb = consts.tile([P, NB], bf16)
        nc.vector.memset(warm_sb[:], 0.0)
        ps_w = ps_bc.tile([P, NB], f32, tag="bc")  # dummy psum, never consumed
        for _ in range(14):
            nc.tensor.matmul(
                ps_w[:], lhsT=warm_sb[:, :P], rhs=warm_sb[:], start=True, stop=True
            )

        # --- startup: block 0's input + the first-needed weights go first, on
        # separate queues, as single contiguous-row transfers.  Weight tiles
        # are m-major ([P, m*(KT*P) + k*P]) so a "half" covers m-chunks 0-3
        # for ALL k and stage N can start before the second half lands ---
        xt_prefetch = {}
        xt_prefetch[0] = load_xt(blocks[0][1], blocks[0][2], eng=nc.sync)
        w1_sb = consts.tile([P, KL * HIDDEN], mmdt)
        for half in range(2):
            nc.scalar.dma_start(
                out=w1_sb[:, half * (KL * HIDDEN) // 2 : (half + 1) * (KL * HIDDEN) // 2],
                in_=w1d[:, half * (KL * HIDDEN) // 2 : (half + 1) * (KL * HIDDEN) // 2],
            )
        b1_sb = consts.tile([P, MH], f32)
        nc.gpsimd.dma_start(out=b1_sb[:], in_=b1d[:])
        b2_sb = consts.tile([P, MH], f32)
        nc.gpsimd.dma_start(out=b2_sb[:], in_=b2d[:])
        # rank-1 constants live at partitions {32,64} = the row groups that
        # consume them in the packed tail slot
        c2t = consts.tile([65, TYPES * OUT], mmdt)
        nc.gpsimd.dma_start(out=c2t[32:33, :], in_=c2d[:])
        nc.gpsimd.dma_start(out=c2t[64:65, :], in_=c2d[:])
        c1t = consts.tile([65, TYPES * OUT], mmdt)
        if use_c1:
            nc.gpsimd.dma_start(out=c1t[32:33, :], in_=c1d[:])
            nc.gpsimd.dma_start(out=c1t[64:65, :], in_=c1d[:])
        onesr = consts.tile([1, P], mmdt)  # lhsT for the rsig broadcast (row group 0)
        nc.vector.memset(onesr[:], 1.0)

        if len(blocks) > 1:
            xt_prefetch[1] = load_xt(blocks[1][1], blocks[1][2], eng=nc.sync)

        w2_sb = consts.tile([P, KH * HIDDEN], mmdt)
        Q2 = KH * HIDDEN // 4
        nc.sync.dma_start(out=w2_sb[:, :Q2], in_=w2d[:, :Q2])
        whp_sb = consts.tile([P, TYPES * KH * OUT], mmdt)
        t0_first = blocks[0][0] if blocks else 0
        type_order = [t0_first] + [t for t in range(TYPES) if t != t0_first]
        nc.scalar.dma_start(
            out=whp_sb[:, t0_first * KH * OUT : (t0_first + 1) * KH * OUT],
            in_=whpd[:, t0_first * KH * OUT : (t0_first + 1) * KH * OUT],
        )
        for q in range(1, 4):
            nc.sync.dma_start(
                out=w2_sb[:, q * Q2 : (q + 1) * Q2], in_=w2d[:, q * Q2 : (q + 1) * Q2]
            )
        if len(blocks) > 2:
            xt_prefetch[2] = load_xt(blocks[2][1], blocks[2][2], eng=nc.sync)
        for t in type_order[1:]:
            nc.scalar.dma_start(
                out=whp_sb[:, t * KH * OUT : (t + 1) * KH * OUT],
                in_=whpd[:, t * KH * OUT : (t + 1) * KH * OUT],
            )

        # ones at lhsT cols {0,32,64}: the stats matmuls emit their column
        # sums at partitions 0/32/64 simultaneously (0: var chain; 32/64:
        # the rank-1 rhs row groups)
        ones65 = consts.tile([P, 65], bf16)
        nc.vector.memset(ones65[:], 0.0)
        for cc in (0, 32, 64):
            nc.vector.memset(ones65[:, cc : cc + 1], 1.0)
        eps_c = consts.tile([P, 1], f32)
        nc.vector.memset(eps_c[:], LN_EPS)
        act_warm = consts.tile([1, 1], f32)
        nc.scalar.activation(act_warm[:], eps_c[0:1, :], AF.Sqrt)

        # --- per-block pipeline (software-pipelined: the LN-dependent DVE/ACT
        # tail of block b runs while block b+1's matmuls keep the PE hot) ---

        def emit_tail(t, c0, nb, ph_list, negmu_t, sv_t, rsig_t):
            # rank-1 corrections + rsig broadcast, packed into one PE slot via
            # row groups 0/1/2 (lhsT+rhs at partitions 0/32/64, distinct banks)
            nc.tensor.matmul(
                ph_list[0][:, :nb],
                lhsT=c2t[32:33, t * OUT : t * OUT + P],
                rhs=negmu_t[32:33, :nb],
                start=False,
                stop=not use_c1,
            )
            nc.tensor.matmul(
                ph_list[1][:, :nb],
                lhsT=c2t[64:65, t * OUT + P : t * OUT + 2 * P],
                rhs=negmu_t[64:65, :nb],
                start=False,
                stop=not use_c1,
            )
            ps_a = ps_bc.tile([P, NB], f32, tag="bc")
            nc.tensor.matmul(
                ps_a[:, :nb], lhsT=onesr[:], rhs=rsig_t[0:1, :nb],
                start=True, stop=True,
            )
            if use_c1:
                nc.tensor.matmul(
                    ph_list[0][:, :nb],
                    lhsT=c1t[32:33, t * OUT : t * OUT + P],
                    rhs=sv_t[32:33, :nb],
                    start=False,
                    stop=True,
                )
                nc.tensor.matmul(
                    ph_list[1][:, :nb],
                    lhsT=c1t[64:65, t * OUT + P : t * OUT + 2 * P],
                    rhs=sv_t[64:65, :nb],
                    start=False,
                    stop=True,
                )
            a_sb = ab_pool.tile([P, NB], f32, tag="a")
            nc.scalar.activation(a_sb[:, :nb], ps_a[:, :nb], AF.Identity)
            out_sb = out_pool.tile([P, MO * NB], f32, tag="out")
            for mc in range(MO):
                nc.vector.tensor_mul(
                    out_sb[:, mc * NB : mc * NB + nb], ph_list[mc][:, :nb],
                    a_sb[:, :nb],
                )
                nc.sync.dma_start(
                    out=outd[mc * P : (mc + 1) * P, c0 : c0 + nb],
                    in_=out_sb[:, mc * NB : mc * NB + nb],
                )

        import functools

        pending = []
        for bi, (t, c0, nb) in enumerate(blocks):
            xt_t = xt_prefetch.pop(bi, None)
            if xt_t is None:
                xt_t = load_xt(c0, nb)

            # stage 1: h1^T = relu(W1^T x + b1)   [HIDDEN, nb]
            h1_t = h1_pool.tile([P, MH * NB], mmdt, tag="h1")
            for m in range(MH):
                ps = ps_mlp.tile([P, NB], f32, tag="ps_mlp")
                for k in range(KL):
                    nc.tensor.matmul(
                        ps[:, :nb],
                        lhsT=w1_sb[:, m * (KL * P) + k * P : m * (KL * P) + (k + 1) * P],
                        rhs=xt_t[:, k * nb : (k + 1) * nb],
                        start=(k == 0),
                        stop=(k == KL - 1),
                    )
                nc.vector.tensor_scalar(
                    h1_t[:, m * NB : m * NB + nb],
                    ps[:, :nb],
                    b1_sb[:, m : m + 1],
                    0.0,
                    op0=mybir.AluOpType.add,
                    op1=mybir.AluOpType.max,
                )

            # the previous block's deferred LN tail slots in here: its PE
            # inputs (negmu/sv/rsig) became ready while this block's stage 1
            # ran, so the packed rank-1 slot never stalls the PE
            if pending:
                pending.pop(0)()

            # stage 2: h2^T = W2^T h1 + b2; squares ride along per chunk
            h2_t = h2_pool.tile([P, MH * NB], mmdt, tag="h2")
            sq_t = sq_pool.tile([P, MH * NB], bf16, tag="sq")
            for m in range(MH):
                ps = ps_mlp.tile([P, NB], f32, tag="ps_mlp")
                for k in range(KH):
                    nc.tensor.matmul(
                        ps[:, :nb],
                        lhsT=w2_sb[:, m * (KH * P) + k * P : m * (KH * P) + (k + 1) * P],
                        rhs=h1_t[:, k * NB : k * NB + nb],
                        start=(k == 0),
                        stop=(k == KH - 1),
                    )
                nc.scalar.activation(
                    h2_t[:, m * NB : m * NB + nb],
                    ps[:, :nb],
                    AF.Identity,
                    bias=b2_sb[:, m : m + 1],
                )
                nc.vector.tensor_mul(
                    sq_t[:, m * NB : m * NB + nb],
                    cv(h2_t[:, m * NB : m * NB + nb]),
                    cv(h2_t[:, m * NB : m * NB + nb]),
                )

            # head matmuls: only need h2, so they keep the PE hot while the
            # stats chain below runs on DVE/GpSimd/ACT
            ph_list = []
            for mc in range(MO):
                ph = ps_head.tile([P, NB], f32, tag="head")
                for k in range(KH):
                    nc.tensor.matmul(
                        ph[:, :nb],
                        lhsT=whp_sb[
                            :,
                            (t * KH + k) * OUT + mc * P : (t * KH + k) * OUT
                            + (mc + 1) * P,
                        ],
                        rhs=h2_t[:, k * NB : k * NB + nb],
                        start=(k == 0),
                        stop=False,
                    )
                ph_list.append(ph)

            # LN stats: pairwise-add tile pairs on DVE (8 chunks -> 1), then
            # partition sums on GpSimd; the Q7 daisy chain leaves the sums on
            # every partition, so downstream ACT/DVE ops on full [128,nb]
            # tiles ARE the partition-broadcast (same cost as one row)
            hs_t = hs_pool.tile([P, (MH // 2) * NB], bf16, tag="hs")
            qs_t = qs_pool.tile([P, (MH // 2) * NB], bf16, tag="qs")
            for k in range(MH // 2):
                nc.vector.tensor_add(
                    hs_t[:, k * NB : k * NB + nb],
                    cv(h2_t[:, 2 * k * NB : 2 * k * NB + nb]),
                    cv(h2_t[:, (2 * k + 1) * NB : (2 * k + 1) * NB + nb]),
                )
            for k in range(MH // 4):
                nc.vector.tensor_add(
                    hs_t[:, k * NB : k * NB + nb],
                    hs_t[:, 2 * k * NB : 2 * k * NB + nb],
                    hs_t[:, (2 * k + 1) * NB : (2 * k + 1) * NB + nb],
                )
            nc.vector.tensor_add(
                hs_t[:, :nb], hs_t[:, :nb], hs_t[:, NB : NB + nb]
            )
            ps_s = ps_stat.tile([65, NB], f32, tag="stat")
            nc.tensor.matmul(
                ps_s[:, :nb], lhsT=ones65[:], rhs=hs_t[:, :nb],
                start=True, stop=True,
            )
            for k in range(MH // 2):
                nc.vector.tensor_add(
                    qs_t[:, k * NB : k * NB + nb],
                    sq_t[:, 2 * k * NB : 2 * k * NB + nb],
                    sq_t[:, (2 * k + 1) * NB : (2 * k + 1) * NB + nb],
                )
            for k in range(MH // 4):
                nc.vector.tensor_add(
                    qs_t[:, k * NB : k * NB + nb],
                    qs_t[:, 2 * k * NB : 2 * k * NB + nb],
                    qs_t[:, (2 * k + 1) * NB : (2 * k + 1) * NB + nb],
                )
            nc.vector.tensor_add(
                qs_t[:, :nb], qs_t[:, :nb], qs_t[:, NB : NB + nb]
            )
            ps_q = ps_stat.tile([65, NB], f32, tag="stat")
            nc.tensor.matmul(
                ps_q[:, :nb], lhsT=ones65[:], rhs=qs_t[:, :nb],
                start=True, stop=True,
            )

            # var chain at partition {0} FIRST (rsig is the deepest chain, so
            # it lands before negmu and the whole rank-1 trio becomes ready at
            # once -> Tile keeps the three row-group matmuls adjacent),
            # then negmu at partitions {32,64} (rank-1 rhs)
            nrow = 65 if use_c1 else 1
            musq_t = rv_pool.tile([65, NB], f32, tag="musq")
            nc.scalar.activation(
                musq_t[0:nrow, :nb], ps_s[0:nrow, :nb], AF.Square,
                scale=1.0 / HIDDEN,
            )
            varv_t = rv_pool.tile([65, NB], f32, tag="varv")
            nc.scalar.activation(
                varv_t[0:nrow, :nb], ps_q[0:nrow, :nb], AF.Identity,
                scale=1.0 / HIDDEN,
            )
            nc.vector.tensor_sub(
                varv_t[0:nrow, :nb], varv_t[0:nrow, :nb], musq_t[0:nrow, :nb]
            )
            if use_c1:
                sv_t = rv_pool.tile([65, NB], mmdt, tag="sv")
                nc.scalar.activation(
                    sv_t[:, :nb], varv_t[:, :nb], AF.Sqrt, bias=eps_c[0:65, :]
                )
            else:
                sv_t = None
            svf_t = rv_pool.tile([1, NB], f32, tag="svf")
            nc.scalar.activation(
                svf_t[:, :nb], varv_t[0:1, :nb], AF.Sqrt, bias=eps_c[0:1, :]
            )
            rsf_t = rv_pool.tile([1, NB], f32, tag="rsf")
            nc.vector.reciprocal_approx_fast(rsf_t[:, :nb], svf_t[:, :nb])
            rsig_t = rv_pool.tile([1, NB], mmdt, tag="rsig")
            nc.scalar.activation(rsig_t[:, :nb], rsf_t[:, :nb], AF.Identity)
            negmu_t = rv_pool.tile([65, NB], mmdt, tag="negmu")
            nc.scalar.activation(
                negmu_t[:, :nb], ps_s[:, :nb], AF.Identity,
                scale=-1.0 / HIDDEN,
            )

            pending.append(functools.partial(
                emit_tail, t, c0, nb, ph_list, negmu_t, sv_t, rsig_t
            ))

        for pf in pending:
            pf()

    nc.compile()
    return nc


def _tf32(x):
    """Round fp32 to TF32 (10-bit mantissa, round-to-nearest-even)."""
    u = np.ascontiguousarray(x, dtype=np.float32).view(np.uint32).copy()
    lsb = (u >> np.uint32(13)) & np.uint32(1)
    u += np.uint32(0x0FFF) + lsb
    u &= np.uint32(0xFFFFE000)
    return u.view(np.float32)


def _tile_cols(a, kt):
    """[kt*P, C] -> [P, kt*C] with col index = k*C + c (the SBUF layout)."""
    kp, C = a.shape
    assert kp == kt * P
    return np.ascontiguousarray(
        a.reshape(kt, P, C).transpose(1, 0, 2).reshape(P, kt * C)
    )


def _tile_cols_mmajor(a, kt):
    """[kt*P, mt*P] -> [P, mt*kt*P] with col index = m*(kt*P) + k*P + pp,
    so a contiguous column range covers a run of m-chunks for ALL k."""
    kp, C = a.shape
    assert kp == kt * P and C % P == 0
    mt = C // P
    return np.ascontiguousarray(
        a.reshape(kt, P, mt, P).transpose(1, 2, 0, 3).reshape(P, mt * kt * P)
    )


def prep_inputs(node_latent, w1, b1, w2, b2, ln_gamma, ln_beta, head_w, head_b,
                caps, idx_by_type, mm_bf16=True):
    """Build the 8 per-core input maps (everything pre-tiled to SBUF layout)."""
    if mm_bf16:
        import ml_dtypes

        cast = lambda a: np.asarray(a, dtype=np.float32).astype(ml_dtypes.bfloat16)
    else:
        cast = _tf32
    whp = np.asarray(ln_gamma)[:, None] * np.asarray(head_w)  # [T, H, OUT]
    whpp = np.concatenate(
        [_tile_cols(cast(whp[t]), KH) for t in range(TYPES)], axis=1
    )  # [P, T*KH*OUT]
    c1 = cast(np.asarray(ln_beta @ head_w + head_b)).reshape(1, TYPES * OUT)
    c2 = cast(np.asarray(ln_gamma @ head_w)).reshape(1, TYPES * OUT)
    w1p = _tile_cols_mmajor(cast(w1), KL)  # [P, MH*KL*P], m-major
    w2p = _tile_cols_mmajor(cast(w2), KH)  # [P, MH*KH*P], m-major
    b1r = np.ascontiguousarray(np.asarray(b1).reshape(MH, P).T).astype(np.float32)
    b2r = np.ascontiguousarray(np.asarray(b2).reshape(MH, P).T).astype(np.float32)
    R = sum(caps)
    blocks = _blocks_from_caps(caps)
    node_latent = np.asarray(node_latent, dtype=np.float32)
    in_maps = []
    for c in range(N_CORES):
        xc = np.zeros((R, LATENT), np.float32)
        off = 0
        for tt in range(TYPES):
            idx = idx_by_type[tt][c]
            xc[off : off + len(idx)] = node_latent[idx]
            off += caps[tt]
        xcb = cast(xc)
        xtp = np.empty((P, KL * R), dtype=xcb.dtype)
        for (_t, c0, nb) in blocks:
            xtp[:, KL * c0 : KL * (c0 + nb)] = (
                xcb[c0 : c0 + nb, :].reshape(nb, KL, P).transpose(2, 1, 0)
                .reshape(P, KL * nb)
            )
        in_maps.append(
            {
                "xtp": xtp,
                "w1p": w1p,
                "w2p": w2p,
                "whpp": whpp,
                "b1r": b1r,
                "b2r": b2r,
                "c1r": c1,
                "c2r": c2,
            }
        )
    return in_maps


def unpack_outputs(results, caps, idx_by_type, n_rows):
    out = np.empty((n_rows, OUT), np.float32)
    for c in range(N_CORES):
        oc = results[c]["out"]  # [OUT, R]
        off = 0
        for tt in range(TYPES):
            idx = idx_by_type[tt][c]
            out[idx] = oc[:, off : off + len(idx)].T
            off += caps[tt]
    return out


def kernel(node_latent, node_types, w1, b1, w2, b2, ln_gamma, ln_beta, head_w, head_b):
    from concourse.bass_utils import run_bass_kernel_spmd

    node_latent = np.asarray(node_latent, dtype=np.float32)
    node_types = np.asarray(node_types)
    blocks, R, caps, idx_by_type = plan(node_types)
    use_c1 = bool(np.any(np.asarray(ln_beta @ head_w + head_b)))
    nc = build_program(blocks, R, use_c1=use_c1, mm_bf16=MM_BF16)
    in_maps = prep_inputs(
        node_latent, w1, b1, w2, b2, ln_gamma, ln_beta, head_w, head_b,
        caps, idx_by_type, mm_bf16=MM_BF16,
    )
    res = run_bass_kernel_spmd(nc, in_maps, core_ids=list(range(N_CORES)))
    return unpack_outputs(res.results, caps, idx_by_type, node_latent.shape[0])


# revision 46
# speedup vs baseline: 1.1938x; 1.1938x over previous
"""Trainium2 Bass kernel for nn_AdaptiveDecoder (shared MLP + hard-routed type heads).

Strategy:
  * Host: sort nodes by type; split each type's count over 8 cores with minimal
    padding (per-type cap = ceil(count/8) rounded to 4) -> every core sees the
    SAME static layout of type-pure node-column blocks, so the compiled SPMD
    program bakes in the block->head mapping and the device does zero routing.
  * Device: activations stay transposed ([feature, nodes]) so the three matmul
    stages chain without transposes, and the PE runs NOTHING but the GEMMs
    (112 cycles per node column = the bf16 roofline):
      - LayerNorm column sums run on GpSimd (partition_all_reduce over the
        DVE-pre-reduced sum / sum-of-squares tiles).  The Q7 daisy chain
        leaves the result on ALL partitions, so the ACT mean/var chain on
        full [128,nb] tiles yields partition-broadcast -mu and 1/sigma for
        free (engines are partition-parallel: a [128,n] op costs the same as
        [1,n]).
      - The mean correction and 1/sigma scaling fold into two fused DVE ops
        per output chunk: out = ((negmu_b * c2_col) + head_psum) * rsig_b.
        gamma is folded into the head weights host-side; beta/head_b (when
        nonzero) are a per-partition constant added by one more DVE op.
  * All DRAM inputs are pre-tiled on the host into their exact SBUF layouts so
    every load is one dma_start with multi-KB contiguous rows.  This matters
    beyond startup: fine-grained strided input DMAs contend with the PE's rhs
    SBUF streaming port and were measured to stretch matmul pacing 260ns ->
    213ns per 512 columns when fixed.
  * Matmuls run bf16 (full PE rate, half the SBUF traffic of f32r).
  * First/last blocks are split small (256/128 cols) to shrink the block-0
    pipeline-fill bubble and the end-of-kernel LN drain.
"""

import sys

sys.path.insert(0, "/opt/trn_rl_repo")

from contextlib import ExitStack

import numpy as np

N_CORES = 8
LATENT, HIDDEN, OUT, TYPES = 512, 1024, 256, 3
P = 128
NB = 512  # node columns per block (PSUM f32 bank limit)
KL = LATENT // P  # 4 k-tiles, stage 1
KH = HIDDEN // P  # 8 k-tiles, stage 2 / head
MH = HIDDEN // P  # 8 m-chunks of hidden
MO = OUT // P  # 2 m-chunks of head output
LN_EPS = 1e-5
MM_BF16 = True


def _caps_from_counts(counts):
    caps = []
    for tt in range(TYPES):
        cap = -(-int(counts[tt]) // N_CORES)  # ceil
        cap = -(-cap // 4) * 4  # round to 4 cols (keeps DMA rows 8B-aligned)
        caps.append(cap)
    return caps


def _blocks_from_caps(caps):
    """Type-pure blocks tiling [0, R). Remainders split so blocks stay >=256;
    the overall first block is 256 (smaller pipeline-fill bubble) and the
    overall last block is 128 (shorter LN drain after the final matmul)."""
    blocks = []
    off = 0
    for tt in range(TYPES):
        cols = caps[tt]
        j = 0
        while j < cols:
            rem = cols - j
            if rem >= 2 * NB:
                nb = NB
            elif rem > NB:
                nb = -(-((rem + 1) // 2) // 4) * 4
            else:
                nb = rem
            blocks.append((tt, off + j, nb))
            j += nb
        off += cols
    if blocks and blocks[0][2] >= NB:
        t, c0, nb = blocks[0]
        blocks[0:1] = [(t, c0, 256), (t, c0 + 256, nb - 256)]
    if blocks and blocks[-1][2] >= 256:
        t, c0, nb = blocks[-1]
        blocks[-1:] = [(t, c0, nb - 128), (t, c0 + nb - 128, 128)]
    return blocks


def plan(node_types, pad_odd=True):
    """Host-side layout plan shared by all cores.

    Returns (blocks, R, caps, idx_by_type) where idx_by_type[t][c] is the array
    of original row indices of type t assigned to core c.
    """
    node_types = np.asarray(node_types)
    counts = np.bincount(node_types, minlength=TYPES)
    caps = _caps_from_counts(counts)
    idx_by_type = []
    order = np.argsort(node_types, kind="stable")
    starts = np.concatenate([[0], np.cumsum(counts)])
    for tt in range(TYPES):
        idx_t = order[starts[tt] : starts[tt + 1]]
        base, rem = divmod(int(counts[tt]), N_CORES)
        parts, o = [], 0
        for c in range(N_CORES):
            n = base + (1 if c < rem else 0)
            parts.append(idx_t[o : o + n])
            o += n
        idx_by_type.append(parts)
    R = sum(caps)
    blocks = _blocks_from_caps(caps)
    return blocks, R, caps, idx_by_type


def build_program(blocks, R, use_c1=True, mm_bf16=True):
    """blocks: list of (type_idx, col_offset, n_cols); R: node columns per core."""
    import concourse.mybir as mybir
    import concourse.tile as tile
    from concourse import bacc, bass_isa

    dt = mybir.dt
    f32, f32r, bf16 = dt.float32, dt.float32r, dt.bfloat16
    mmdt = bf16 if mm_bf16 else f32r
    AF = mybir.ActivationFunctionType
    ALU = mybir.AluOpType

    nc = bacc.Bacc("TRN2", target_bir_lowering=False, debug=False, num_devices=N_CORES)

    xtd = nc.dram_tensor("xtp", [P, KL * R], mmdt, kind="ExternalInput").ap()
    w1d = nc.dram_tensor("w1p", [P, KL * HIDDEN], mmdt, kind="ExternalInput").ap()
    w2d = nc.dram_tensor("w2p", [P, KH * HIDDEN], mmdt, kind="ExternalInput").ap()
    whpd = nc.dram_tensor("whpp", [P, TYPES * KH * OUT], mmdt, kind="ExternalInput").ap()
    b1d = nc.dram_tensor("b1r", [P, MH], f32, kind="ExternalInput").ap()
    b2d = nc.dram_tensor("b2r", [P, MH], f32, kind="ExternalInput").ap()
    c1d = nc.dram_tensor("c1r", [1, TYPES * OUT], mmdt, kind="ExternalInput").ap()
    c2d = nc.dram_tensor("c2r", [1, TYPES * OUT], mmdt, kind="ExternalInput").ap()
    outd = nc.dram_tensor("out", [OUT, R], f32, kind="ExternalOutput").ap()

    def cv(ap):  # engine-facing view of an mm-dtype tile
        return ap if mm_bf16 else ap.bitcast(f32)

    with tile.TileContext(nc) as tc, ExitStack() as ctx:
        consts = ctx.enter_context(tc.tile_pool(name="consts", bufs=1))
        xt_pool = ctx.enter_context(tc.tile_pool(name="xt", bufs=3))
        h1_pool = ctx.enter_context(tc.tile_pool(name="h1", bufs=2))
        h2_pool = ctx.enter_context(tc.tile_pool(name="h2", bufs=2))
        sq_pool = ctx.enter_context(tc.tile_pool(name="sq", bufs=1))
        hs_pool = ctx.enter_context(tc.tile_pool(name="hs", bufs=2))
        qs_pool = ctx.enter_context(tc.tile_pool(name="qs", bufs=2))
        rv_pool = ctx.enter_context(tc.tile_pool(name="rv", bufs=2))
        ab_pool = ctx.enter_context(tc.tile_pool(name="ab", bufs=2))
        out_pool = ctx.enter_context(tc.tile_pool(name="outp", bufs=2))
        ps_mlp = ctx.enter_context(tc.tile_pool(name="ps_mlp", bufs=3, space="PSUM"))
        ps_head = ctx.enter_context(tc.tile_pool(name="ps_head", bufs=2, space="PSUM"))
        ps_stat = ctx.enter_context(tc.tile_pool(name="ps_stat", bufs=2, space="PSUM"))
        ps_bc = ctx.enter_context(tc.tile_pool(name="ps_bc", bufs=1, space="PSUM"))

        # gpsimd is reserved for the LN partition reductions (plus one-time
        # startup DMAs); recurring xt loads round-robin on the HWDGE queues
        dma_engines = [nc.sync, nc.scalar]
        dma_rr = [0]

        def dma(out, in_):
            eng = dma_engines[dma_rr[0] % len(dma_engines)]
            dma_rr[0] += 1
            eng.dma_start(out=out, in_=in_)

        def load_xt(c0, nb, eng=None):
            xt_t = xt_pool.tile([P, KL * NB], mmdt, tag="xt")
            if eng is None:
                dma(xt_t[:, : KL * nb], xtd[:, KL * c0 : KL * (c0 + nb)])
            else:
                eng.dma_start(out=xt_t[:, : KL * nb], in_=xtd[:, KL * c0 : KL * (c0 + nb)])
            return xt_t

        # --- PE warm-up: the HAM clock-gate needs ~3.4us of sustained PE
        # activity to release full clock; burn the DMA-wait window on dummy
        # matmuls over a memset scratch tile so the first real matmul runs at
        # 2.4 GHz instead of 1.2 ---
        warm_sb = consts.tile([P, NB], bf16)
        nc.vector.memset(warm_sb[:], 0.0)
        ps_w = ps_bc.tile([P, NB], f32, tag="bc")  # dummy psum, never consumed
        for _ in range(14):
            nc.tensor.matmul(
                ps_w[:], lhsT=warm_sb[:, :P], rhs=warm_sb[:], start=True, stop=True
            )

        # --- startup: block 0's input + the first-needed weights go first, on
        # separate queues, as single contiguous-row transfers.  Weight tiles
        # are m-major ([P, m*(KT*P) + k*P]) so a "half" covers m-chunks 0-3
        # for ALL k and stage N can start before the second half lands ---
        xt_prefetch = {}
        xt_prefetch[0] = load_xt(blocks[0][1], blocks[0][2], eng=nc.sync)
        w1_sb = consts.tile([P, KL * HIDDEN], mmdt)
        for half in range(2):
            nc.scalar.dma_start(
                out=w1_sb[:, half * (KL * HIDDEN) // 2 : (half + 1) * (KL * HIDDEN) // 2],
                in_=w1d[:, half * (KL * HIDDEN) // 2 : (half + 1) * (KL * HIDDEN) // 2],
            )
        b1_sb = consts.tile([P, MH], f32)
        nc.gpsimd.dma_start(out=b1_sb[:], in_=b1d[:])
        b2_sb = consts.tile([P, MH], f32)
        nc.gpsimd.dma_start(out=b2_sb[:], in_=b2d[:])
        # rank-1 constants live at partitions {32,64} = the row groups that
        # consume them in the packed tail slot
        c2t = consts.tile([65, TYPES * OUT], mmdt)
        nc.gpsimd.dma_start(out=c2t[32:33, :], in_=c2d[:])
        nc.gpsimd.dma_start(out=c2t[64:65, :], in_=c2d[:])
        c1t = consts.tile([65, TYPES * OUT], mmdt)
        if use_c1:
            nc.gpsimd.dma_start(out=c1t[32:33, :], in_=c1d[:])
            nc.gpsimd.dma_start(out=c1t[64:65, :], in_=c1d[:])
        onesr = consts.tile([1, P], mmdt)  # lhsT for the rsig broadcast (row group 0)
        nc.vector.memset(onesr[:], 1.0)

        if len(blocks) > 1:
            xt_prefetch[1] = load_xt(blocks[1][1], blocks[1][2], eng=nc.sync)

        w2_sb = consts.tile([P, KH * HIDDEN], mmdt)
        Q2 = KH * HIDDEN // 4
        nc.sync.dma_start(out=w2_sb[:, :Q2], in_=w2d[:, :Q2])
        whp_sb = consts.tile([P, TYPES * KH * OUT], mmdt)
        t0_first = blocks[0][0] if blocks else 0
        type_order = [t0_first] + [t for t in range(TYPES) if t != t0_first]
        nc.scalar.dma_start(
            out=whp_sb[:, t0_first * KH * OUT : (t0_first + 1) * KH * OUT],
            in_=whpd[:, t0_first * KH * OUT : (t0_first + 1) * KH * OUT],
        )
        for q in range(1, 4):
            nc.sync.dma_start(
                out=w2_sb[:, q * Q2 : (q + 1) * Q2], in_=w2d[:, q * Q2 : (q + 1) * Q2]
            )
        if len(blocks) > 2:
            xt_prefetch[2] = load_xt(blocks[2][1], blocks[2][2], eng=nc.sync)
        for t in type_order[1:]:
            nc.scalar.dma_start(
                out=whp_sb[:, t * KH * OUT : (t + 1) * KH * OUT],
                in_=whpd[:, t * KH * OUT : (t + 1) * KH * OUT],
            )

        # ones at lhsT cols {0,32,64}: the stats matmuls emit their column
        # sums at partitions 0/32/64 simultaneously (0: var chain; 32/64:
        # the rank-1 rhs row groups)
        ones65 = consts.tile([P, 65], bf16)
        nc.vector.memset(ones65[:], 0.0)
        for cc in (0, 32, 64):
            nc.vector.memset(ones65[:, cc : cc + 1], 1.0)
        eps_c = consts.tile([P, 1], f32)
        nc.vector.memset(eps_c[:], LN_EPS)
        act_warm = consts.tile([1, 1], f32)
        nc.scalar.activation(act_warm[:], eps_c[0:1, :], AF.Sqrt)

        # --- per-block pipeline (software-pipelined: the LN-dependent DVE/ACT
        # tail of block b runs while block b+1's matmuls keep the PE hot) ---

        def emit_tail(t, c0, nb, ph_list, negmu_t, sv_t, rsig_t):
            # rank-1 corrections + rsig broadcast, packed into one PE slot via
            # row groups 0/1/2 (lhsT+rhs at partitions 0/32/64, distinct banks)
            nc.tensor.matmul(
                ph_list[0][:, :nb],
                lhsT=c2t[32:33, t * OUT : t * OUT + P],
                rhs=negmu_t[32:33, :nb],
                start=False,
                stop=not use_c1,
            )
            nc.tensor.matmul(
                ph_list[1][:, :nb],
                lhsT=c2t[64:65, t * OUT + P : t * OUT + 2 * P],
                rhs=negmu_t[64:65, :nb],
                start=False,
                stop=not use_c1,
            )
            ps_a = ps_bc.tile([P, NB], f32, tag="bc")
            nc.tensor.matmul(
                ps_a[:, :nb], lhsT=onesr[:], rhs=rsig_t[0:1, :nb],
                start=True, stop=True,
            )
            if use_c1:
                nc.tensor.matmul(
                    ph_list[0][:, :nb],
                    lhsT=c1t[32:33, t * OUT : t * OUT + P],
                    rhs=sv_t[32:33, :nb],
                    start=False,
                    stop=True,
                )
                nc.tensor.matmul(
                    ph_list[1][:, :nb],
                    lhsT=c1t[64:65, t * OUT + P : t * OUT + 2 * P],
                    rhs=sv_t[64:65, :nb],
                    start=False,
                    stop=True,
                )
            a_sb = ab_pool.tile([P, NB], f32, tag="a")
            nc.scalar.activation(a_sb[:, :nb], ps_a[:, :nb], AF.Identity)
            out_sb = out_pool.tile([P, MO * NB], f32, tag="out")
            for mc in range(MO):
                nc.vector.tensor_mul(
                    out_sb[:, mc * NB : mc * NB + nb], ph_list[mc][:, :nb],
                    a_sb[:, :nb],
                )
                nc.sync.dma_start(
                    out=outd[mc * P : (mc + 1) * P, c0 : c0 + nb],
                    in_=out_sb[:, mc * NB : mc * NB + nb],
                )

        import functools

        pending = []
        for bi, (t, c0, nb) in enumerate(blocks):
            xt_t = xt_prefetch.pop(bi, None)
            if xt_t is None:
                xt_t = load_xt(c0, nb)

            # stage 1: h1^T = relu(W1^T x + b1)   [HIDDEN, nb]
            h1_t = h1_pool.tile([P, MH * NB], mmdt, tag="h1")
            for m in range(MH):
                ps = ps_mlp.tile([P, NB], f32, tag="ps_mlp")
                for k in range(KL):
                    nc.tensor.matmul(
                        ps[:, :nb],
                        lhsT=w1_sb[:, m * (KL * P) + k * P : m * (KL * P) + (k + 1) * P],
                        rhs=xt_t[:, k * nb : (k + 1) * nb],
                        start=(k == 0),
                        stop=(k == KL - 1),
                    )
                nc.vector.tensor_scalar(
                    h1_t[:, m * NB : m * NB + nb],
                    ps[:, :nb],
                    b1_sb[:, m : m + 1],
                    0.0,
                    op0=mybir.AluOpType.add,
                    op1=mybir.AluOpType.max,
                )

            # the previous block's deferred LN tail slots in here: its PE
            # inputs (negmu/sv/rsig) became ready while this block's stage 1
            # ran, so the packed rank-1 slot never stalls the PE
            if pending:
                pending.pop(0)()

            # stage 2: h2^T = W2^T h1 + b2; squares ride along per chunk
            h2_t = h2_pool.tile([P, MH * NB], mmdt, tag="h2")
            sq_t = sq_pool.tile([P, MH * NB], bf16, tag="sq")
            for m in range(MH):
                ps = ps_mlp.tile([P, NB], f32, tag="ps_mlp")
                for k in range(KH):
                    nc.tensor.matmul(
                        ps[:, :nb],
                        lhsT=w2_sb[:, m * (KH * P) + k * P : m * (KH * P) + (k + 1) * P],
                        rhs=h1_t[:, k * NB : k * NB + nb],
                        start=(k == 0),
                        stop=(k == KH - 1),
                    )
                nc.scalar.activation(
                    h2_t[:, m * NB : m * NB + nb],
                    ps[:, :nb],
                    AF.Identity,
                    bias=b2_sb[:, m : m + 1],
                )
                nc.vector.tensor_mul(
                    sq_t[:, m * NB : m * NB + nb],
                    cv(h2_t[:, m * NB : m * NB + nb]),
                    cv(h2_t[:, m * NB : m * NB + nb]),
                )

            # head matmuls: only need h2, so they keep the PE hot while the
            # stats chain below runs on DVE/GpSimd/ACT
            ph_list = []
            for mc in range(MO):
                ph = ps_head.tile([P, NB], f32, tag="head")
                for k in range(KH):
                    nc.tensor.matmul(
                        ph[:, :nb],
                        lhsT=whp_sb[
                            :,
                            (t * KH + k) * OUT + mc * P : (t * KH + k) * OUT
                            + (mc + 1) * P,
                        ],
                        rhs=h2_t[:, k * NB : k * NB + nb],
                        start=(k == 0),
                        stop=False,
                    )
                ph_list.append(ph)

            # LN stats: pairwise-add tile pairs on DVE (8 chunks -> 1), then
            # partition sums on GpSimd; the Q7 daisy chain leaves the sums on
            # every partition, so downstream ACT/DVE ops on full [128,nb]
            # tiles ARE the partition-broadcast (same cost as one row)
            hs_t = hs_pool.tile([P, (MH // 2) * NB], bf16, tag="hs")
            qs_t = qs_pool.tile([P, (MH // 2) * NB], bf16, tag="qs")
            for k in range(MH // 2):
                nc.vector.tensor_add(
                    hs_t[:, k * NB : k * NB + nb],
                    cv(h2_t[:, 2 * k * NB : 2 * k * NB + nb]),
                    cv(h2_t[:, (2 * k + 1) * NB : (2 * k + 1) * NB + nb]),
                )
            for k in range(MH // 4):
                nc.vector.tensor_add(
                    hs_t[:, k * NB : k * NB + nb],
                    hs_t[:, 2 * k * NB : 2 * k * NB + nb],
                    hs_t[:, (2 * k + 1) * NB : (2 * k + 1) * NB + nb],
                )
            nc.vector.tensor_add(
                hs_t[:, :nb], hs_t[:, :nb], hs_t[:, NB : NB + nb]
            )
            ps_s = ps_stat.tile([65, NB], f32, tag="stat")
            nc.tensor.matmul(
                ps_s[:, :nb], lhsT=ones65[:], rhs=hs_t[:, :nb],
                start=True, stop=True,
            )
            for k in range(MH // 2):
                nc.vector.tensor_add(
                    qs_t[:, k * NB : k * NB + nb],
                    sq_t[:, 2 * k * NB : 2 * k * NB + nb],
                    sq_t[:, (2 * k + 1) * NB : (2 * k + 1) * NB + nb],
                )
            for k in range(MH // 4):
                nc.vector.tensor_add(
                    qs_t[:, k * NB : k * NB + nb],
                    qs_t[:, 2 * k * NB : 2 * k * NB + nb],
                    qs_t[:, (2 * k + 1) * NB : (2 * k + 1) * NB + nb],
                )
            nc.vector.tensor_add(
                qs_t[:, :nb], qs_t[:, :nb], qs_t[:, NB : NB + nb]
            )
            ps_q = ps_stat.tile([65, NB], f32, tag="stat")
            nc.tensor.matmul(
                ps_q[:, :nb], lhsT=ones65[:], rhs=qs_t[:, :nb],
                start=True, stop=True,
            )

            # negmu at partitions {32,64} (rank-1 rhs) first -- the c2 pair
            # sits earliest in the PE FIFO, so its input must land earliest
            negmu_t = rv_pool.tile([65, NB], mmdt, tag="negmu")
            nc.scalar.activation(
                negmu_t[:, :nb], ps_s[:, :nb], AF.Identity,
                scale=-1.0 / HIDDEN,
            )
            nrow = 65 if use_c1 else 1
            musq_t = rv_pool.tile([65, NB], f32, tag="musq")
            nc.scalar.activation(
                musq_t[0:nrow, :nb], ps_s[0:nrow, :nb], AF.Square,
                scale=1.0 / HIDDEN,
            )
            varv_t = rv_pool.tile([65, NB], f32, tag="varv")
            nc.scalar.activation(
                varv_t[0:nrow, :nb], ps_q[0:nrow, :nb], AF.Identity,
                scale=1.0 / HIDDEN,
            )
            nc.vector.tensor_sub(
                varv_t[0:nrow, :nb], varv_t[0:nrow, :nb], musq_t[0:nrow, :nb]
            )
            if use_c1:
                sv_t = rv_pool.tile([65, NB], mmdt, tag="sv")
                nc.scalar.activation(
                    sv_t[:, :nb], varv_t[:, :nb], AF.Sqrt, bias=eps_c[0:65, :]
                )
            else:
                sv_t = None
            svf_t = rv_pool.tile([1, NB], f32, tag="svf")
            nc.scalar.activation(
                svf_t[:, :nb], varv_t[0:1, :nb], AF.Sqrt, bias=eps_c[0:1, :]
            )
            rsf_t = rv_pool.tile([1, NB], f32, tag="rsf")
            nc.vector.reciprocal_approx_fast(rsf_t[:, :nb], svf_t[:, :nb])
            rsig_t = rv_pool.tile([1, NB], mmdt, tag="rsig")
            nc.scalar.activation(rsig_t[:, :nb], rsf_t[:, :nb], AF.Identity)

            pending.append(functools.partial(
                emit_tail, t, c0, nb, ph_list, negmu_t, sv_t, rsig_t
            ))

        for pf in pending:
            pf()

    nc.compile()
    return nc


def _tf32(x):
    """Round fp32 to TF32 (10-bit mantissa, round-to-nearest-even)."""
    u = np.ascontiguousarray(x, dtype=np.float32).view(np.uint32).copy()
    lsb = (u >> np.uint32(13)) & np.uint32(1)
    u += np.uint32(0x0FFF) + lsb
    u &= np.uint32(0xFFFFE000)
    return u.view(np.float32)


def _tile_cols(a, kt):
    """[kt*P, C] -> [P, kt*C] with col index = k*C + c (the SBUF layout)."""
    kp, C = a.shape
    assert kp == kt * P
    return np.ascontiguousarray(
        a.reshape(kt, P, C).transpose(1, 0, 2).reshape(P, kt * C)
    )


def _tile_cols_mmajor(a, kt):
    """[kt*P, mt*P] -> [P, mt*kt*P] with col index = m*(kt*P) + k*P + pp,
    so a contiguous column range covers a run of m-chunks for ALL k."""
    kp, C = a.shape
    assert kp == kt * P and C % P == 0
    mt = C // P
    return np.ascontiguousarray(
        a.reshape(kt, P, mt, P).transpose(1, 2, 0, 3).reshape(P, mt * kt * P)
    )


def prep_inputs(node_latent, w1, b1, w2, b2, ln_gamma, ln_beta, head_w, head_b,
                caps, idx_by_type, mm_bf16=True):
    """Build the 8 per-core input maps (everything pre-tiled to SBUF layout)."""
    if mm_bf16:
        import ml_dtypes

        cast = lambda a: np.asarray(a, dtype=np.float32).astype(ml_dtypes.bfloat16)
    else:
        cast = _tf32
    whp = np.asarray(ln_gamma)[:, None] * np.asarray(head_w)  # [T, H, OUT]
    whpp = np.concatenate(
        [_tile_cols(cast(whp[t]), KH) for t in range(TYPES)], axis=1
    )  # [P, T*KH*OUT]
    c1 = cast(np.asarray(ln_beta @ head_w + head_b)).reshape(1, TYPES * OUT)
    c2 = cast(np.asarray(ln_gamma @ head_w)).reshape(1, TYPES * OUT)
    w1p = _tile_cols_mmajor(cast(w1), KL)  # [P, MH*KL*P], m-major
    w2p = _tile_cols_mmajor(cast(w2), KH)  # [P, MH*KH*P], m-major
    b1r = np.ascontiguousarray(np.asarray(b1).reshape(MH, P).T).astype(np.float32)
    b2r = np.ascontiguousarray(np.asarray(b2).reshape(MH, P).T).astype(np.float32)
    R = sum(caps)
    blocks = _blocks_from_caps(caps)
    node_latent = np.asarray(node_latent, dtype=np.float32)
    in_maps = []
    for c in range(N_CORES):
        xc = np.zeros((R, LATENT), np.float32)
        off = 0
        for tt in range(TYPES):
            idx = idx_by_type[tt][c]
            xc[off : off + len(idx)] = node_latent[idx]
            off += caps[tt]
        xcb = cast(xc)
        xtp = np.empty((P, KL * R), dtype=xcb.dtype)
        for (_t, c0, nb) in blocks:
            xtp[:, KL * c0 : KL * (c0 + nb)] = (
                xcb[c0 : c0 + nb, :].reshape(nb, KL, P).transpose(2, 1, 0)
                .reshape(P, KL * nb)
            )
        in_maps.append(
            {
                "xtp": xtp,
                "w1p": w1p,
                "w2p": w2p,
                "whpp": whpp,
                "b1r": b1r,
                "b2r": b2r,
                "c1r": c1,
                "c2r": c2,
            }
        )
    return in_maps


def unpack_outputs(results, caps, idx_by_type, n_rows):
    out = np.empty((n_rows, OUT), np.float32)
    for c in range(N_CORES):
        oc = results[c]["out"]  # [OUT, R]
        off = 0
        for tt in range(TYPES):
            idx = idx_by_type[tt][c]
            out[idx] = oc[:, off : off + len(idx)].T
            off += caps[tt]
    return out


def kernel(node_latent, node_types, w1, b1, w2, b2, ln_gamma, ln_beta, head_w, head_b):
    from concourse.bass_utils import run_bass_kernel_spmd

    node_latent = np.asarray(node_latent, dtype=np.float32)
    node_types = np.asarray(node_types)
    blocks, R, caps, idx_by_type = plan(node_types)
    use_c1 = bool(np.any(np.asarray(ln_beta @ head_w + head_b)))
    nc = build_program(blocks, R, use_c1=use_c1, mm_bf16=MM_BF16)
    in_maps = prep_inputs(
        node_latent, w1, b1, w2, b2, ln_gamma, ln_beta, head_w, head_b,
        caps, idx_by_type, mm_bf16=MM_BF16,
    )
    res = run_bass_kernel_spmd(nc, in_maps, core_ids=list(range(N_CORES)))
    return unpack_outputs(res.results, caps, idx_by_type, node_latent.shape[0])
